# revision 39
# baseline (speedup 1.0000x reference)
"""MsPoE Llama attention on 8 TRN2 NeuronCores (tensor-parallel over heads).

Strategy (v7: single launch, PE-lean attention, ~586us vs 673us baseline)
-------------------------------------------------------------------------
The reference's head-ordering statistic only needs the LAST pre-RoPE
attention row: srow_h = q_last_h . k_h[s]. By associativity,
srow_h = hs @ (Wk_h^T (Wq_h hs[-1])) — ~0.5 GFLOP, computed on the
host in float64 BEFORE launching (verified to reproduce the reference
head_order exactly). With head_order known up-front, the permuted
per-head RoPE cos/sin caches become plain inputs and the whole module
runs in ONE device launch with q/k/v resident in SBUF.

  PSUM discipline (everything below depends on it): 4 tags x 2 banks,
  each tag's ring rotation matching its freeing order per phase.

  Per core (4 heads), all matmul operands bf16:

  1. QK pass: stream hsT once per sb; wq streams, wk is RESIDENT
     (loaded during sb0 — with both streaming, phase 1 was DMA-starved
     at 74% queue occupancy). Accumulate q/k head-blocks in PSUM,
     then drain: 8 psum->bf16 copies FIRST (alternating ACT/DVE), and
     RoPE's rotate_half as a tiny PE permutation matmul (P2, +/-1)
     back into each just-freed bank — a gpsimd half-multiply here ran
     1.3-1.5us/op and serialized the drain, cascading into the sync
     engine's DMA queue via the trig-ring WAR. The next pass's first
     hs chunk is prefetched before the drain so sb boundaries don't
     stall on DMA. RoPE math: dst = qf*cos + (P2 qf)*sin on DVE.
  2. V pass: stream hsT again, natural-layout V into resident
     v_m [128, 16, 512] bf16. wo loads here, into wk's SBUF slot
     (issued after each sb's hs DMAs — never at the head of the sync
     queue, where its wk WAR-wait would block all later DMA issue).
  3. Attention per (qb outer, head inner): scores^T = rk_chunk^T rq
     per 128-key tile; exp on ACT with a 2^-4 prescale (bias), bf16
     out; causal masking is a 0/1 lower-triangle MULTIPLY on the
     diagonal 128-col block of the exp tile (exact: masked entries
     become 0). The softmax denominator accumulates on the DVE into
     fp16 zacc (2-byte ops, cannot overflow thanks to the prescale;
     the prescale cancels in AV/z normalization). AV accumulates
     unnormalized in PSUM.
  4. Deferred job FIFO, popped 3 PE-ops per kt iteration: per-head z
     ones-matmul + reciprocal (one head late), 1/z PE broadcast +
     attnT normalize (two heads late, so zb never waits the recip
     chain), then o_proj matmuls in et-PAIRS ping-ponging psC's two
     banks. A fixed pop rate (not backlog-proportional) keeps o_proj
     matmuls spaced; excess spills into later query blocks/the tail.

  Host: sum the 8 o_proj partials (f64) -> [1, 2048, 4096].
"""

import os
import sys

import numpy as np

for _p in ("/opt/trn_rl_repo", "/root/.axon_site/_ro/trn_rl_repo"):
    if os.path.isdir(_p) and _p not in sys.path:
        sys.path.append(_p)

import concourse.bass as bass  # noqa: E402
import concourse.tile as tile  # noqa: E402
from concourse import bacc, mybir  # noqa: E402
from concourse import bass_utils  # noqa: E402

import ml_dtypes  # noqa: E402

F32 = mybir.dt.float32
BF16 = mybir.dt.bfloat16
F32R = mybir.dt.float32r
F16 = mybir.dt.float16
NPBF16 = ml_dtypes.bfloat16

B, S, HID, H, D = 1, 2048, 4096, 32, 128
NCORES, HPC = 8, 4          # cores, heads per core
JC = HPC * D                # 512: per-core projection width
KT = HID // 128             # 32 contraction tiles
SB = S // 512               # 4 sequence blocks
EB = 4                      # e-tiles per hs-stream DMA
BASE, MIN_R, MAX_R = 10000.0, 1.0, 3.0
SCALE = 1.0 / float(np.sqrt(D))
EXPB = -4.0 * float(np.log(2.0))   # exp pre-scale 2^-4: fp16 z never overflows

_CACHE = {}
TRACE = False          # set True (e.g. from test.py) to profile the launch
LAST_PROFILE = {}      # filled with BassKernelResults when TRACE is on


def build():
    nc = bacc.Bacc("TRN2", target_bir_lowering=False, debug=False, num_devices=NCORES)
    hsT = nc.dram_tensor("hsT", [HID, S], BF16, kind="ExternalInput").ap()
    wqT = nc.dram_tensor("wqT", [HID, JC], BF16, kind="ExternalInput").ap()
    wkT = nc.dram_tensor("wkT", [HID, JC], BF16, kind="ExternalInput").ap()
    wvT = nc.dram_tensor("wvT", [HID, JC], BF16, kind="ExternalInput").ap()
    woT = nc.dram_tensor("woT", [JC, HID], BF16, kind="ExternalInput").ap()
    cosT = nc.dram_tensor("cosT", [JC, S], BF16, kind="ExternalInput").ap()
    shatT = nc.dram_tensor("shatT", [JC, S], BF16, kind="ExternalInput").ap()
    masks = nc.dram_tensor("masks", [128, 128], BF16, kind="ExternalInput").ap()
    p2rot = nc.dram_tensor("p2rot", [128, 128], BF16, kind="ExternalInput").ap()
    oT = nc.dram_tensor("oT", [HID, S], BF16, kind="ExternalOutput").ap()

    hsT_b = hsT.rearrange("(eb g p) s -> p eb g s", p=128, g=EB)   # [128, 8, EB, S]
    wqT_b = wqT.rearrange("(eb g p) j -> p eb g j", p=128, g=EB)   # [128, 8, EB, JC]
    wkT_b = wkT.rearrange("(eb g p) j -> p eb g j", p=128, g=EB)
    wvT_b = wvT.rearrange("(kt p) j -> p kt j", p=128)             # [128, 32, JC]
    woT_b = woT.rearrange("(jt p) e -> p jt e", p=128)             # [128, 4, HID]
    cosT_b = cosT.rearrange("(h p) s -> p h s", p=128)             # [128, 4, S]
    shatT_b = shatT.rearrange("(h p) s -> p h s", p=128)
    oT_b = oT.rearrange("(et p) s -> p et s", p=128)               # [128, 32, S]
    NB = KT // EB

    with tile.TileContext(nc) as tc:
        with (
            tc.tile_pool(name="wv_res", bufs=1) as wv_res,   # wv resident
            tc.tile_pool(name="wkwo", bufs=1) as wkwo,       # wk then wo (shared slot)
            tc.tile_pool(name="wst", bufs=4) as wst,         # wq streamed chunks
            tc.tile_pool(name="trig", bufs=4) as trig,       # cos/shat per-sb stream
            tc.tile_pool(name="big", bufs=4) as big,         # rq/rk/v/attnT resident
            tc.tile_pool(name="hpool", bufs=3) as hpool,     # hs stream tiles
            tc.tile_pool(name="rtmp", bufs=4) as rtmp,       # RoPE f32 temps
            tc.tile_pool(name="expp", bufs=8) as expp,       # exp/qf tiles bf16
            tc.tile_pool(name="small", bufs=1) as small,
            tc.tile_pool(name="zacc_p", bufs=2) as zacc_p,   # z accumulators f32r
            tc.tile_pool(name="zp", bufs=4) as zp,           # 1/z rows
            tc.tile_pool(name="outp", bufs=4) as outp,
            tc.tile_pool(name="ps", bufs=8, space="PSUM") as ps,
        ):
            rq = big.tile([128, HPC, S], BF16, tag="big", name="rq")
            rk = big.tile([128, HPC, S], BF16, tag="big", name="rk")
            wv_m = wv_res.tile([128, KT, JC], BF16, tag="w", name="wv")
            wk_m = wkwo.tile([128, KT, JC], BF16, tag="wk", name="wk")

            p2m = small.tile([128, 128], BF16)
            nc.sync.dma_start(p2m, p2rot)

            # ---- phase 1: QK projections + fused RoPE ----
            pre1 = {}
            pre2 = {}
            for sb in range(SB):
                ss = slice(sb * 512, (sb + 1) * 512)
                # fixed-lifetime PSUM rings: 4 tags x 2 banks. Each tag's
                # rotation order then matches its freeing order in every phase
                ps_q = [ps.tile([128, 512], F32, tag=f"ps{'AB'[i // 2]}",
                                bufs=2, name=f"psq{sb}_{i}")
                        for i in range(HPC)]
                ps_k = [ps.tile([128, 512], F32, tag=f"ps{'CD'[i // 2]}",
                                bufs=2, name=f"psk{sb}_{i}")
                        for i in range(HPC)]
                cos_t = trig.tile([128, HPC, 512], BF16, tag="t", name=f"cos{sb}")
                shat_t = trig.tile([128, HPC, 512], BF16, tag="t", name=f"shat{sb}")
                for eb in range(NB):
                    # half-tile DMAs land on different queues: halves the
                    # transfer latency each chunk's first matmuls gate on
                    if eb == 0 and sb in pre1:
                        hst, wqs = pre1.pop(sb)   # prefetched last sb
                    else:
                        hst = hpool.tile([128, EB, 512], BF16, tag="h")
                        wqs = wst.tile([128, EB, JC], BF16, tag="w",
                                       name=f"wqs{sb}_{eb}")
                        for g4 in range(EB):
                            nc.sync.dma_start(hst[:, g4:g4 + 1],
                                              hsT_b[:, eb, g4:g4 + 1, ss])
                            nc.sync.dma_start(wqs[:, g4:g4 + 1],
                                              wqT_b[:, eb, g4:g4 + 1])
                    if sb == 0:
                        nc.sync.dma_start(
                            wk_m[:, eb * EB: eb * EB + EB // 2],
                            wkT_b[:, eb, :EB // 2],
                        )
                        nc.sync.dma_start(
                            wk_m[:, eb * EB + EB // 2: (eb + 1) * EB],
                            wkT_b[:, eb, EB // 2:],
                        )
                    if eb == 0:
                        nc.sync.dma_start(cos_t, cosT_b[:, :, ss])
                        nc.sync.dma_start(shat_t, shatT_b[:, :, ss])
                    if eb == NB - 1:
                        # prefetch the NEXT pass's first chunk NOW, ahead of
                        # the wv/trig transfers and the drain in the sync
                        # queue (for sb3 that's phase 2's first chunk)
                        ss2 = slice(((sb + 1) % SB) * 512,
                                    ((sb + 1) % SB + 1) * 512)
                        hst2 = hpool.tile([128, EB, 512], BF16, tag="h")
                        nc.sync.dma_start(hst2[:, :EB // 2],
                                          hsT_b[:, 0, :EB // 2, ss2])
                        nc.sync.dma_start(hst2[:, EB // 2:],
                                          hsT_b[:, 0, EB // 2:, ss2])
                        if sb + 1 < SB:
                            wqs2 = wst.tile([128, EB, JC], BF16, tag="w",
                                            name=f"wqs{sb + 1}_0")
                            nc.sync.dma_start(wqs2[:, :EB // 2],
                                              wqT_b[:, 0, :EB // 2])
                            nc.sync.dma_start(wqs2[:, EB // 2:],
                                              wqT_b[:, 0, EB // 2:])
                            pre1[sb + 1] = (hst2, wqs2)
                        else:
                            pre2[0] = hst2
                    last = eb == NB - 1
                    for g in range(EB):
                        e = eb * EB + g
                        if last and g == EB - 1:
                            # final chunk: all q matmuls first, so the q
                            # banks' drain overlaps the k matmuls
                            for jt in range(HPC):
                                js = slice(jt * 128, (jt + 1) * 128)
                                nc.tensor.matmul(ps_q[jt], wqs[:, g, js],
                                                 hst[:, g], start=False, stop=True)
                            for jt in range(HPC):
                                js = slice(jt * 128, (jt + 1) * 128)
                                nc.tensor.matmul(ps_k[jt], wk_m[:, e, js],
                                                 hst[:, g], start=False, stop=True)
                        else:
                            for jt in range(HPC):
                                js = slice(jt * 128, (jt + 1) * 128)
                                nc.tensor.matmul(
                                    ps_q[jt], wqs[:, g, js], hst[:, g],
                                    start=(e == 0), stop=False,
                                )
                                nc.tensor.matmul(
                                    ps_k[jt], wk_m[:, e, js], hst[:, g],
                                    start=(e == 0), stop=False,
                                )
                # prefetch one quarter of wv per sb
                pcs = slice(sb * 8, (sb + 1) * 8)
                nc.sync.dma_start(wv_m[:, pcs], wvT_b[:, pcs])
                # Drain ALL 8 PSUM banks first (copies alternate ACT/DVE, in
                # the bank order the next sb's matmuls need). RoPE's
                # rotate_half is a tiny PE permutation matmul (P2, +/-1
                # entries) back into the just-freed bank — the gpsimd
                # half-multiply it replaces ran 1.3-1.5us per op and
                # serialized the whole drain tail.
                chains = []
                cp = 0
                if sb == SB - 1:
                    order = [0, 2, 1, 3]   # frees psA,psB,psC,psD round-robin
                else:
                    order = [0, 1, 2, 3]   # next sb needs q0,k0,q1,k1,...
                for jt in order:
                    for ps_list, dst, qk in ((ps_q, rq, "q"), (ps_k, rk, "k")):
                        qf = expp.tile([128, 512], BF16, tag="exp")
                        if cp % 2 == 0:
                            nc.scalar.copy(qf, ps_list[jt])
                        else:
                            nc.vector.tensor_copy(qf, ps_list[jt])
                        cp += 1
                        tagc = ("AB" if qk == "q" else "CD")[jt // 2]
                        chains.append((qf, dst, jt, tagc, qk))
                swqs = []
                for qf, dst, jt, tagc, qk in chains:
                    swq = ps.tile([128, 512], F32, tag=f"ps{tagc}", bufs=2,
                                  name=f"swq{sb}_{qk}{jt}")
                    nc.tensor.matmul(swq, p2m, qf, start=True, stop=True)
                    swqs.append(swq)
                for (qf, dst, jt, tagc, qk), swq in zip(chains, swqs):
                    tmp = rtmp.tile([128, 512], BF16, tag="rt", bufs=2)
                    with nc.allow_low_precision(reason="rope bf16 temp"):
                        nc.vector.tensor_mul(tmp, swq, shat_t[:, jt])
                    t2 = rtmp.tile([128, 512], BF16, tag="rt_t2", bufs=2)
                    nc.vector.tensor_mul(t2, qf, cos_t[:, jt])
                    with nc.allow_low_precision(reason="rope bf16 store"):
                        nc.vector.tensor_add(dst[:, jt, ss], t2, tmp)

            # ---- constants for attention (loaded behind phase 2's stream) ----
            # 0/1 lower-triangle template: multiplies the diagonal 128-col
            # sub-block of each exp tile (replaces the additive -1e35 mask;
            # exp of an unmasked score is finite, then zeroed exactly)
            masks_sb = small.tile([128, 128], BF16)
            nc.sync.dma_start(masks_sb, masks)
            onesf = rtmp.tile([128, 1], F32, tag="ones", bufs=2)
            nc.vector.memset(onesf, 1.0)
            ones_col = small.tile([128, 1], F16)
            nc.vector.tensor_copy(ones_col, onesf)
            onesf_r = rtmp.tile([1, 128], F32, tag="ones", bufs=2)
            nc.vector.memset(onesf_r, 1.0)
            ones_row = small.tile([1, 128], F16)
            nc.vector.tensor_copy(ones_row, onesf_r)
            expb = small.tile([128, 1], F32)
            nc.vector.memset(expb, EXPB)

            # ---- phase 2: V projection (natural layout); wo loads into wk's
            # slot (wk is dead after phase 1) ----
            wo_m = wkwo.tile([128, HPC, HID], BF16, tag="wk", name="wo")
            v_m = big.tile([128, S // 128, JC], BF16, tag="big", name="v")
            for sb in range(SB):
                ss = slice(sb * 512, (sb + 1) * 512)
                ps_v = [ps.tile([128, 512], F32, tag=f"ps{'ABCD'[i]}",
                                bufs=2, name=f"psv{sb}_{i}")
                        for i in range(4)]
                for eb in range(NB):
                    if eb == 0 and sb in pre2:
                        hst = pre2.pop(sb)
                    else:
                        hst = hpool.tile([128, EB, 512], BF16, tag="h")
                        for g4 in range(EB):
                            nc.sync.dma_start(hst[:, g4:g4 + 1],
                                              hsT_b[:, eb, g4:g4 + 1, ss])
                    if eb == NB - 1 and sb + 1 < SB:
                        ss2 = slice((sb + 1) * 512, (sb + 2) * 512)
                        hst2 = hpool.tile([128, EB, 512], BF16, tag="h")
                        nc.sync.dma_start(hst2[:, :EB // 2],
                                          hsT_b[:, 0, :EB // 2, ss2])
                        nc.sync.dma_start(hst2[:, EB // 2:],
                                          hsT_b[:, 0, EB // 2:, ss2])
                        pre2[sb + 1] = hst2
                    for g in range(EB):
                        e = eb * EB + g
                        for t4 in range(4):
                            cs = slice(t4 * 128, (t4 + 1) * 128)
                            nc.tensor.matmul(
                                ps_v[t4], hst[:, g, cs], wv_m[:, e],
                                start=(e == 0), stop=(e == KT - 1),
                            )
                # one quarter of wo per sb, issued AFTER this sb's hst
                # dma_starts so they never queue behind the wk WAR wait
                nc.sync.dma_start(wo_m[:, :, sb * 1024:(sb + 1) * 1024],
                                  woT_b[:, :, sb * 1024:(sb + 1) * 1024])
                for t4 in range(4):
                    if t4 % 2 == 0:
                        nc.scalar.copy(v_m[:, sb * 4 + t4], ps_v[t4])
                    else:
                        nc.vector.tensor_copy(v_m[:, sb * 4 + t4], ps_v[t4])

            # ---- phase 3+4: attention (qb outer) + deferred normalize/o_proj
            attnT = big.tile([128, HPC * SB, 512], BF16, tag="big", name="attnT")
            oo_flip = 0

            def zn_jobs(oqb, zacc, h):
                """z partition-sum + reciprocal, then 1/z broadcast +
                normalize, for one finished head — deferred by one head."""
                zrbox = {}

                def zjob():
                    ps_z = ps.tile([1, 512], F32, tag="psD", bufs=2,
                                   name=f"psz{oqb}_{h}")
                    nc.tensor.matmul(ps_z, ones_col, zacc[:, h],
                                     start=True, stop=True)
                    zf = zp.tile([1, 512], F32, tag="zf", bufs=1)
                    nc.vector.reciprocal_approx_fast(zf, ps_z)
                    zr = zp.tile([1, 512], F16, tag="zr", bufs=2)
                    with nc.allow_low_precision(reason="1/z fp16 bcast"):
                        nc.vector.tensor_copy(zr, zf)
                    zrbox[0] = zr

                def njob():
                    zb = ps.tile([128, 512], F32, tag="psD", bufs=2,
                                 name=f"zb{oqb}_{h}")
                    nc.tensor.matmul(zb, ones_row, zrbox[0],
                                     start=True, stop=True)
                    i16 = h * SB + oqb
                    with nc.allow_low_precision(reason="attn norm bf16"):
                        nc.vector.tensor_tensor(
                            attnT[:, i16], attnT[:, i16], zb,
                            op=mybir.AluOpType.mult,
                        )

                return [zjob, njob]

            def oproj_jobs(oqb):
                """o_proj matmuls in et-PAIRS ping-ponging two PSUM banks
                so consecutive o_proj matmuls never share a bank."""
                jobs = []
                opsum = {}

                def ojob(et, jt):
                    def run():
                        pso = opsum[et % 2]
                        nc.tensor.matmul(
                            pso, wo_m[:, jt, et * 128:(et + 1) * 128],
                            attnT[:, jt * SB + oqb],
                            start=(jt == 0), stop=(jt == HPC - 1),
                        )
                        if jt == HPC - 1:
                            drain(et, pso)
                    return run

                def drain(et, ps_oo):
                    oo = outp.tile([128, 512], BF16, tag="oo", bufs=3)
                    nc.vector.tensor_copy(oo, ps_oo)
                    m0 = oqb * 512
                    nc.sync.dma_start(oT_b[:, et, m0:m0 + 256], oo[:, :256])
                    nc.sync.dma_start(oT_b[:, et, m0 + 256:m0 + 512], oo[:, 256:])

                def mkop(par, et):
                    def run():
                        opsum[par] = ps.tile([128, 512], F32, tag="psC",
                                             bufs=2, name=f"poo{oqb}_{et}")
                    return run

                # o_proj in et pairs ping-ponging psC's two banks
                for pair in range(KT // 2):
                    ea, eb_ = 2 * pair, 2 * pair + 1
                    jobs.append(mkop(0, ea))
                    jobs.append(mkop(1, eb_))
                    for jt in range(HPC):
                        jobs.append(ojob(ea, jt))
                        jobs.append(ojob(eb_, jt))
                return jobs

            pending = []
            njobs_hold = []
            for qb in range(SB):
                nkt = 4 * qb + 4
                zacc = zacc_p.tile([128, HPC, 512], F16, tag="za",
                                   name=f"zacc{qb % 2}")
                for h in range(HPC):
                    # fixed pop rate: a backlog-proportional stride crams
                    # 16+ o_proj matmuls between attention matmuls (2-bank
                    # ping-pong degrades to ~360ns each); excess work just
                    # spills into later query blocks / the tail instead
                    stride = 3 if pending else 0
                    ps_o = ps.tile([128, 512], F32, tag="psB", bufs=2,
                                   name=f"pso{qb}_{h}")

                    def score_exp(kt):
                        # diagonal blocks: columns j < 128*r are fully
                        # masked -> skip them (w = valid width)
                        r = kt - 4 * qb
                        j0 = 128 * r if r > 0 else 0
                        w = 512 - j0
                        qsw_ = slice(qb * 512 + j0, (qb + 1) * 512)
                        ps_s = ps.tile([128, w], F32, tag="psA", bufs=2,
                                       name=f"pss{qb}_{h}_{kt}")
                        nc.tensor.matmul(
                            ps_s, rk[:, h, kt * 128: (kt + 1) * 128],
                            rq[:, h, qsw_], start=True, stop=True,
                        )
                        ext = expp.tile([128, w], BF16, tag="exp")
                        # exp is pre-scaled by 2^-4 so the fp16 z
                        # accumulator cannot overflow; the factor cancels
                        # exactly in AV/z normalization
                        nc.scalar.activation(
                            ext, ps_s, mybir.ActivationFunctionType.Exp,
                            scale=SCALE, bias=expb,
                        )
                        if r >= 0:
                            with nc.allow_low_precision(reason="mask 0/1"):
                                nc.vector.tensor_mul(
                                    ext[:, :128], ext[:, :128], masks_sb
                                )
                        return ext, j0

                    # software pipeline: score/exp one kt ahead of AV, so
                    # the PE never waits on the mask+exp latency chain
                    nxt = score_exp(0)
                    for kt in range(nkt):
                        ext, j0 = nxt
                        if kt + 1 < nkt:
                            nxt = score_exp(kt + 1)
                        nc.tensor.matmul(
                            ps_o[:, j0:512],
                            v_m[:, kt, h * 128: (h + 1) * 128], ext,
                            start=(kt == 0), stop=(kt == nkt - 1),
                        )
                        # softmax denominator on the DVE (kt==0 is always
                        # full width, so plain copy initializes)
                        if kt == 0:
                            nc.vector.tensor_copy(zacc[:, h], ext)
                        else:
                            nc.vector.tensor_add(
                                zacc[:, h, j0:512], zacc[:, h, j0:512], ext
                            )
                        for _ in range(stride):
                            if pending:
                                pending.pop(0)()
                    # drain unnormalized AV rows (frees ps_o)
                    i16 = h * SB + qb
                    nc.vector.tensor_copy(attnT[:, i16], ps_o)
                    # this head's z pops during the NEXT head; its normalize
                    # one head later still, so the 1/z broadcast never waits
                    # on the reciprocal's DVE latency
                    zj, nj = zn_jobs(qb, zacc, h)
                    if h > 0:
                        pending.append(njobs_hold.pop(0))
                    pending.append(zj)
                    njobs_hold.append(nj)
                pending.append(njobs_hold.pop(0))
                pending.extend(oproj_jobs(qb))
            while pending:            # final query block's normalize + o_proj
                pending.pop(0)()

    nc.compile()
    return nc


def _get_nc():
    if "S" not in _CACHE:
        _CACHE["S"] = build()
    return _CACHE["S"]


def _causal_mask_templates():
    # 0/1 lower-triangle: within the diagonal 128x128 block of every
    # exp tile, key p attends query column j iff p <= j
    p = np.arange(128)[:, None]
    j = np.arange(128)[None, :]
    return np.ascontiguousarray(np.where(p > j, 0.0, 1.0).astype(np.float32))


def _rope_cache_np():
    # mirrors reference._rope_cache in float32
    inv_freq = (1.0 / (BASE ** (np.arange(0, D, 2, dtype=np.float32) / np.float32(D)))).astype(np.float32)
    ratio = (MIN_R + (MAX_R - MIN_R) * (np.arange(H, dtype=np.float32) / np.float32(H))).astype(np.float32)
    t = (np.arange(S, dtype=np.float32)[None, :] / ratio[:, None]).astype(np.float32)
    freqs = (t[:, :, None] * inv_freq[None, None, :]).astype(np.float32)
    emb = np.concatenate([freqs, freqs], axis=-1)
    return np.cos(emb).astype(np.float32), np.sin(emb).astype(np.float32)


def _head_order(hs, Wq, Wk):
    """Exact head-outlier ordering from the last pre-RoPE attention row,
    computed in f64 on the host: srow_h = hs @ (Wk_h^T (Wq_h hs[-1]))."""
    hs64 = hs.astype(np.float64)
    q_last = hs64[-1] @ Wq.T.astype(np.float64)                 # [HID]
    Wk64 = Wk.astype(np.float64)
    Wall = np.empty((HID, H), np.float64)
    for h in range(H):
        rows = slice(h * D, (h + 1) * D)
        Wall[:, h] = Wk64[rows, :].T @ q_last[rows]
    srow = (hs64 @ Wall).T                                      # [H, S]
    sc = srow * SCALE
    m = sc.max(axis=-1, keepdims=True)
    e = np.exp(sc - m)
    aw = e / e.sum(axis=-1, keepdims=True)
    avg = aw.mean(axis=-1, keepdims=True)
    cnt = (aw > 3.0 * avg).sum(axis=-1)
    outlier = (-(cnt / np.float32(S))).astype(np.float32)
    return np.argsort(outlier, kind="stable")


def kernel(hidden_states, position_ids, Wq, Wk, Wv, Wo):
    hs = np.asarray(hidden_states, dtype=np.float32)[0]        # [S, HID]
    pos = np.asarray(position_ids).astype(np.int64)[0]         # [S]
    Wq = np.asarray(Wq, dtype=np.float32)
    Wk = np.asarray(Wk, dtype=np.float32)
    Wv = np.asarray(Wv, dtype=np.float32)
    Wo = np.asarray(Wo, dtype=np.float32)

    # ---- host: head order (exact control flow), permuted RoPE caches ----
    head_order = _head_order(hs, Wq, Wk)
    cos, sin = _rope_cache_np()
    cos_o = cos[head_order][:, pos, :]                         # [H, S, D]
    sin_o = sin[head_order][:, pos, :]
    masks = _causal_mask_templates()
    # rotate_half as a matmul: sw[m] = -x[m+64] (m<64), +x[m-64] (m>=64)
    p2rot_m = np.zeros((128, 128), np.float32)
    p2rot_m[np.arange(64) + 64, np.arange(64)] = -1.0
    p2rot_m[np.arange(64), np.arange(64) + 64] = 1.0
    p2rot_m = p2rot_m.astype(NPBF16)

    hsT = np.ascontiguousarray(hs.T).astype(NPBF16)            # [HID, S] bf16

    nc = _get_nc()
    in_maps = []
    for c in range(NCORES):
        rows = slice(c * JC, (c + 1) * JC)
        ct = np.ascontiguousarray(
            np.concatenate([cos_o[c * HPC + i].T for i in range(HPC)], axis=0)
        )  # [JC, S]
        # plain sin (rotate_half's swap+sign lives in the P2 matmul)
        st = np.concatenate(
            [sin_o[c * HPC + i].T for i in range(HPC)], axis=0
        )
        in_maps.append(
            {
                "hsT": hsT,
                "wqT": np.ascontiguousarray(Wq[rows, :].T).astype(NPBF16),
                "wkT": np.ascontiguousarray(Wk[rows, :].T).astype(NPBF16),
                "wvT": np.ascontiguousarray(Wv[rows, :].T).astype(NPBF16),
                "woT": np.ascontiguousarray(Wo[:, rows].T).astype(NPBF16),
                "cosT": ct.astype(NPBF16),
                "shatT": np.ascontiguousarray(st).astype(NPBF16),
                "masks": masks.astype(NPBF16),
                "p2rot": p2rot_m,
            }
        )
    res = bass_utils.run_bass_kernel_spmd(
        nc, in_maps, core_ids=list(range(NCORES)), trace=TRACE
    )
    if TRACE:
        LAST_PROFILE["S"] = res

    # ---- host: unshard (sum o_proj partials) ----
    acc = np.zeros((HID, S), np.float64)
    for c in range(NCORES):
        acc += res.results[c]["oT"].astype(np.float64)
    return np.ascontiguousarray(acc.T)[None, :, :].astype(np.float32)


# revision 40
# speedup vs baseline: 1.0135x; 1.0135x over previous
"""MsPoE Llama attention on 8 TRN2 NeuronCores (tensor-parallel over heads).

Strategy (v7: single launch, PE-lean attention, ~586us vs 673us baseline)
-------------------------------------------------------------------------
The reference's head-ordering statistic only needs the LAST pre-RoPE
attention row: srow_h = q_last_h . k_h[s]. By associativity,
srow_h = hs @ (Wk_h^T (Wq_h hs[-1])) — ~0.5 GFLOP, computed on the
host in float64 BEFORE launching (verified to reproduce the reference
head_order exactly). With head_order known up-front, the permuted
per-head RoPE cos/sin caches become plain inputs and the whole module
runs in ONE device launch with q/k/v resident in SBUF.

  PSUM discipline (everything below depends on it): 4 tags x 2 banks,
  each tag's ring rotation matching its freeing order per phase.

  Per core (4 heads), all matmul operands bf16:

  1. QK pass: stream hsT once per sb; wq streams, wk is RESIDENT
     (loaded during sb0 — with both streaming, phase 1 was DMA-starved
     at 74% queue occupancy). Accumulate q/k head-blocks in PSUM,
     then drain: 8 psum->bf16 copies FIRST (alternating ACT/DVE), and
     RoPE's rotate_half as a tiny PE permutation matmul (P2, +/-1)
     back into each just-freed bank — a gpsimd half-multiply here ran
     1.3-1.5us/op and serialized the drain, cascading into the sync
     engine's DMA queue via the trig-ring WAR. The next pass's first
     hs chunk is prefetched before the drain so sb boundaries don't
     stall on DMA. RoPE math: dst = qf*cos + (P2 qf)*sin on DVE.
  2. V pass: stream hsT again, natural-layout V into resident
     v_m [128, 16, 512] bf16. wo loads here, into wk's SBUF slot
     (issued after each sb's hs DMAs — never at the head of the sync
     queue, where its wk WAR-wait would block all later DMA issue).
  3. Attention per (qb outer, head inner): scores^T = rk_chunk^T rq
     per 128-key tile; exp on ACT with a 2^-4 prescale (bias), bf16
     out; causal masking is a 0/1 lower-triangle MULTIPLY on the
     diagonal 128-col block of the exp tile (exact: masked entries
     become 0). The softmax denominator accumulates on the DVE into
     fp16 zacc (2-byte ops, cannot overflow thanks to the prescale;
     the prescale cancels in AV/z normalization). AV accumulates
     unnormalized in PSUM.
  4. Deferred job FIFO, popped 3 PE-ops per kt iteration: per-head z
     ones-matmul + reciprocal (one head late), 1/z PE broadcast +
     attnT normalize (two heads late, so zb never waits the recip
     chain), then o_proj matmuls in et-PAIRS ping-ponging psC's two
     banks. A fixed pop rate (not backlog-proportional) keeps o_proj
     matmuls spaced; excess spills into later query blocks/the tail.

  Host: sum the 8 o_proj partials (f64) -> [1, 2048, 4096].
"""

import os
import sys

import numpy as np

for _p in ("/opt/trn_rl_repo", "/root/.axon_site/_ro/trn_rl_repo"):
    if os.path.isdir(_p) and _p not in sys.path:
        sys.path.append(_p)

import concourse.bass as bass  # noqa: E402
import concourse.tile as tile  # noqa: E402
from concourse import bacc, mybir  # noqa: E402
from concourse import bass_utils  # noqa: E402

import ml_dtypes  # noqa: E402

F32 = mybir.dt.float32
BF16 = mybir.dt.bfloat16
F32R = mybir.dt.float32r
F16 = mybir.dt.float16
NPBF16 = ml_dtypes.bfloat16

B, S, HID, H, D = 1, 2048, 4096, 32, 128
NCORES, HPC = 8, 4          # cores, heads per core
JC = HPC * D                # 512: per-core projection width
KT = HID // 128             # 32 contraction tiles
SB = S // 512               # 4 sequence blocks
EB = 4                      # e-tiles per hs-stream DMA
BASE, MIN_R, MAX_R = 10000.0, 1.0, 3.0
SCALE = 1.0 / float(np.sqrt(D))
EXPB = -4.0 * float(np.log(2.0))   # exp pre-scale 2^-4: fp16 z never overflows

_CACHE = {}
TRACE = False          # set True (e.g. from test.py) to profile the launch
LAST_PROFILE = {}      # filled with BassKernelResults when TRACE is on


def build():
    nc = bacc.Bacc("TRN2", target_bir_lowering=False, debug=False, num_devices=NCORES)
    hsT = nc.dram_tensor("hsT", [HID, S], BF16, kind="ExternalInput").ap()
    wqT = nc.dram_tensor("wqT", [HID, JC], BF16, kind="ExternalInput").ap()
    wkT = nc.dram_tensor("wkT", [HID, JC], BF16, kind="ExternalInput").ap()
    wvT = nc.dram_tensor("wvT", [HID, JC], BF16, kind="ExternalInput").ap()
    woT = nc.dram_tensor("woT", [JC, HID], BF16, kind="ExternalInput").ap()
    cosT = nc.dram_tensor("cosT", [JC, S], BF16, kind="ExternalInput").ap()
    shatT = nc.dram_tensor("shatT", [JC, S], BF16, kind="ExternalInput").ap()
    masks = nc.dram_tensor("masks", [128, 128], BF16, kind="ExternalInput").ap()
    p2rot = nc.dram_tensor("p2rot", [128, 128], BF16, kind="ExternalInput").ap()
    oT = nc.dram_tensor("oT", [HID, S], BF16, kind="ExternalOutput").ap()

    hsT_b = hsT.rearrange("(eb g p) s -> p eb g s", p=128, g=EB)   # [128, 8, EB, S]
    wqT_b = wqT.rearrange("(eb g p) j -> p eb g j", p=128, g=EB)   # [128, 8, EB, JC]
    wkT_b = wkT.rearrange("(eb g p) j -> p eb g j", p=128, g=EB)
    wvT_b = wvT.rearrange("(kt p) j -> p kt j", p=128)             # [128, 32, JC]
    woT_b = woT.rearrange("(jt p) e -> p jt e", p=128)             # [128, 4, HID]
    cosT_b = cosT.rearrange("(h p) s -> p h s", p=128)             # [128, 4, S]
    shatT_b = shatT.rearrange("(h p) s -> p h s", p=128)
    oT_b = oT.rearrange("(et p) s -> p et s", p=128)               # [128, 32, S]
    NB = KT // EB

    with tile.TileContext(nc) as tc:
        with (
            tc.tile_pool(name="wv_res", bufs=1) as wv_res,   # wv resident
            tc.tile_pool(name="wkwo", bufs=1) as wkwo,       # wk then wo (shared slot)
            tc.tile_pool(name="wst", bufs=4) as wst,         # wq streamed chunks
            tc.tile_pool(name="trig", bufs=4) as trig,       # cos/shat per-sb stream
            tc.tile_pool(name="big", bufs=4) as big,         # rq/rk/v/attnT resident
            tc.tile_pool(name="hpool", bufs=3) as hpool,     # hs stream tiles
            tc.tile_pool(name="rtmp", bufs=4) as rtmp,       # RoPE f32 temps
            tc.tile_pool(name="expp", bufs=8) as expp,       # exp/qf tiles bf16
            tc.tile_pool(name="small", bufs=1) as small,
            tc.tile_pool(name="zacc_p", bufs=2) as zacc_p,   # z accumulators f32r
            tc.tile_pool(name="zp", bufs=4) as zp,           # 1/z rows
            tc.tile_pool(name="outp", bufs=4) as outp,
            tc.tile_pool(name="ps", bufs=8, space="PSUM") as ps,
        ):
            rq = big.tile([128, HPC, S], BF16, tag="big", name="rq")
            rk = big.tile([128, HPC, S], BF16, tag="big", name="rk")
            wv_m = wv_res.tile([128, KT, JC], BF16, tag="w", name="wv")
            wk_m = wkwo.tile([128, KT, JC], BF16, tag="wk", name="wk")

            p2m = small.tile([128, 128], BF16)
            nc.sync.dma_start(p2m, p2rot)

            # ---- phase 1: QK projections + fused RoPE ----
            pre1 = {}
            pre2 = {}
            for sb in range(SB):
                ss = slice(sb * 512, (sb + 1) * 512)
                # fixed-lifetime PSUM rings: 4 tags x 2 banks. Each tag's
                # rotation order then matches its freeing order in every phase
                ps_q = [ps.tile([128, 512], F32, tag=f"ps{'AB'[i // 2]}",
                                bufs=2, name=f"psq{sb}_{i}")
                        for i in range(HPC)]
                ps_k = [ps.tile([128, 512], F32, tag=f"ps{'CD'[i // 2]}",
                                bufs=2, name=f"psk{sb}_{i}")
                        for i in range(HPC)]
                cos_t = trig.tile([128, HPC, 512], BF16, tag="t", name=f"cos{sb}")
                shat_t = trig.tile([128, HPC, 512], BF16, tag="t", name=f"shat{sb}")
                for eb in range(NB):
                    # half-tile DMAs land on different queues: halves the
                    # transfer latency each chunk's first matmuls gate on
                    if eb == 0 and sb in pre1:
                        hst, wqs = pre1.pop(sb)   # prefetched last sb
                    else:
                        hst = hpool.tile([128, EB, 512], BF16, tag="h")
                        nc.sync.dma_start(hst[:, :EB // 2], hsT_b[:, eb, :EB // 2, ss])
                        wqs = wst.tile([128, EB, JC], BF16, tag="w",
                                       name=f"wqs{sb}_{eb}")
                        nc.sync.dma_start(wqs[:, :EB // 2], wqT_b[:, eb, :EB // 2])
                        nc.sync.dma_start(hst[:, EB // 2:], hsT_b[:, eb, EB // 2:, ss])
                        nc.sync.dma_start(wqs[:, EB // 2:], wqT_b[:, eb, EB // 2:])
                    if sb == 0:
                        nc.sync.dma_start(
                            wk_m[:, eb * EB: eb * EB + EB // 2],
                            wkT_b[:, eb, :EB // 2],
                        )
                        nc.sync.dma_start(
                            wk_m[:, eb * EB + EB // 2: (eb + 1) * EB],
                            wkT_b[:, eb, EB // 2:],
                        )
                    if eb == 0:
                        nc.sync.dma_start(cos_t, cosT_b[:, :, ss])
                        nc.sync.dma_start(shat_t, shatT_b[:, :, ss])
                    if eb == NB - 1:
                        # prefetch the NEXT pass's first chunk NOW, ahead of
                        # the wv/trig transfers and the drain in the sync
                        # queue (for sb3 that's phase 2's first chunk)
                        ss2 = slice(((sb + 1) % SB) * 512,
                                    ((sb + 1) % SB + 1) * 512)
                        hst2 = hpool.tile([128, EB, 512], BF16, tag="h")
                        nc.sync.dma_start(hst2[:, :EB // 2],
                                          hsT_b[:, 0, :EB // 2, ss2])
                        nc.sync.dma_start(hst2[:, EB // 2:],
                                          hsT_b[:, 0, EB // 2:, ss2])
                        if sb + 1 < SB:
                            wqs2 = wst.tile([128, EB, JC], BF16, tag="w",
                                            name=f"wqs{sb + 1}_0")
                            nc.sync.dma_start(wqs2[:, :EB // 2],
                                              wqT_b[:, 0, :EB // 2])
                            nc.sync.dma_start(wqs2[:, EB // 2:],
                                              wqT_b[:, 0, EB // 2:])
                            pre1[sb + 1] = (hst2, wqs2)
                        else:
                            pre2[0] = hst2
                    last = eb == NB - 1
                    for g in range(EB):
                        e = eb * EB + g
                        if last and g == EB - 1:
                            # final chunk: all q matmuls first, so the q
                            # banks' drain overlaps the k matmuls
                            for jt in range(HPC):
                                js = slice(jt * 128, (jt + 1) * 128)
                                nc.tensor.matmul(ps_q[jt], wqs[:, g, js],
                                                 hst[:, g], start=False, stop=True)
                            for jt in range(HPC):
                                js = slice(jt * 128, (jt + 1) * 128)
                                nc.tensor.matmul(ps_k[jt], wk_m[:, e, js],
                                                 hst[:, g], start=False, stop=True)
                        else:
                            for jt in range(HPC):
                                js = slice(jt * 128, (jt + 1) * 128)
                                nc.tensor.matmul(
                                    ps_q[jt], wqs[:, g, js], hst[:, g],
                                    start=(e == 0), stop=False,
                                )
                                nc.tensor.matmul(
                                    ps_k[jt], wk_m[:, e, js], hst[:, g],
                                    start=(e == 0), stop=False,
                                )
                # prefetch one quarter of wv per sb
                pcs = slice(sb * 8, (sb + 1) * 8)
                nc.sync.dma_start(wv_m[:, pcs], wvT_b[:, pcs])
                # Drain ALL 8 PSUM banks first (copies alternate ACT/DVE, in
                # the bank order the next sb's matmuls need). RoPE's
                # rotate_half is a tiny PE permutation matmul (P2, +/-1
                # entries) back into the just-freed bank — the gpsimd
                # half-multiply it replaces ran 1.3-1.5us per op and
                # serialized the whole drain tail.
                chains = []
                cp = 0
                if sb == SB - 1:
                    order = [0, 2, 1, 3]   # frees psA,psB,psC,psD round-robin
                else:
                    order = [0, 1, 2, 3]   # next sb needs q0,k0,q1,k1,...
                for jt in order:
                    for ps_list, dst, qk in ((ps_q, rq, "q"), (ps_k, rk, "k")):
                        qf = expp.tile([128, 512], BF16, tag="exp")
                        if cp % 2 == 0:
                            nc.scalar.copy(qf, ps_list[jt])
                        else:
                            nc.vector.tensor_copy(qf, ps_list[jt])
                        cp += 1
                        tagc = ("AB" if qk == "q" else "CD")[jt // 2]
                        chains.append((qf, dst, jt, tagc, qk))
                swqs = []
                for qf, dst, jt, tagc, qk in chains:
                    swq = ps.tile([128, 512], F32, tag=f"ps{tagc}", bufs=2,
                                  name=f"swq{sb}_{qk}{jt}")
                    nc.tensor.matmul(swq, p2m, qf, start=True, stop=True)
                    swqs.append(swq)
                for (qf, dst, jt, tagc, qk), swq in zip(chains, swqs):
                    tmp = rtmp.tile([128, 512], BF16, tag="rt", bufs=2)
                    with nc.allow_low_precision(reason="rope bf16 temp"):
                        nc.vector.tensor_mul(tmp, swq, shat_t[:, jt])
                    t2 = rtmp.tile([128, 512], BF16, tag="rt_t2", bufs=2)
                    nc.vector.tensor_mul(t2, qf, cos_t[:, jt])
                    with nc.allow_low_precision(reason="rope bf16 store"):
                        nc.vector.tensor_add(dst[:, jt, ss], t2, tmp)

            # ---- constants for attention (loaded behind phase 2's stream) ----
            # 0/1 lower-triangle template: multiplies the diagonal 128-col
            # sub-block of each exp tile (replaces the additive -1e35 mask;
            # exp of an unmasked score is finite, then zeroed exactly)
            masks_sb = small.tile([128, 128], BF16)
            nc.sync.dma_start(masks_sb, masks)
            onesf = rtmp.tile([128, 1], F32, tag="ones", bufs=2)
            nc.vector.memset(onesf, 1.0)
            ones_col = small.tile([128, 1], F16)
            nc.vector.tensor_copy(ones_col, onesf)
            onesf_r = rtmp.tile([1, 128], F32, tag="ones", bufs=2)
            nc.vector.memset(onesf_r, 1.0)
            ones_row = small.tile([1, 128], F16)
            nc.vector.tensor_copy(ones_row, onesf_r)
            expb = small.tile([128, 1], F32)
            nc.vector.memset(expb, EXPB)

            # ---- phase 2: V projection (natural layout); wo loads into wk's
            # slot (wk is dead after phase 1) ----
            wo_m = wkwo.tile([128, HPC, HID], BF16, tag="wk", name="wo")
            v_m = big.tile([128, S // 128, JC], BF16, tag="big", name="v")
            for sb in range(SB):
                ss = slice(sb * 512, (sb + 1) * 512)
                ps_v = [ps.tile([128, 512], F32, tag=f"ps{'ABCD'[i]}",
                                bufs=2, name=f"psv{sb}_{i}")
                        for i in range(4)]
                for eb in range(NB):
                    if eb == 0 and sb in pre2:
                        hst = pre2.pop(sb)
                    else:
                        hst = hpool.tile([128, EB, 512], BF16, tag="h")
                        nc.sync.dma_start(hst[:, :EB // 2],
                                          hsT_b[:, eb, :EB // 2, ss])
                        nc.sync.dma_start(hst[:, EB // 2:],
                                          hsT_b[:, eb, EB // 2:, ss])
                    if eb == NB - 1 and sb + 1 < SB:
                        ss2 = slice((sb + 1) * 512, (sb + 2) * 512)
                        hst2 = hpool.tile([128, EB, 512], BF16, tag="h")
                        nc.sync.dma_start(hst2[:, :EB // 2],
                                          hsT_b[:, 0, :EB // 2, ss2])
                        nc.sync.dma_start(hst2[:, EB // 2:],
                                          hsT_b[:, 0, EB // 2:, ss2])
                        pre2[sb + 1] = hst2
                    for g in range(EB):
                        e = eb * EB + g
                        for t4 in range(4):
                            cs = slice(t4 * 128, (t4 + 1) * 128)
                            nc.tensor.matmul(
                                ps_v[t4], hst[:, g, cs], wv_m[:, e],
                                start=(e == 0), stop=(e == KT - 1),
                            )
                # one quarter of wo per sb, issued AFTER this sb's hst
                # dma_starts so they never queue behind the wk WAR wait
                nc.sync.dma_start(wo_m[:, :, sb * 1024:(sb + 1) * 1024],
                                  woT_b[:, :, sb * 1024:(sb + 1) * 1024])
                for t4 in range(4):
                    if t4 % 2 == 0:
                        nc.scalar.copy(v_m[:, sb * 4 + t4], ps_v[t4])
                    else:
                        nc.vector.tensor_copy(v_m[:, sb * 4 + t4], ps_v[t4])

            # ---- phase 3+4: attention (qb outer) + deferred normalize/o_proj
            attnT = big.tile([128, HPC * SB, 512], BF16, tag="big", name="attnT")
            oo_flip = 0

            def zn_jobs(oqb, zacc, h):
                """z partition-sum + reciprocal, then 1/z broadcast +
                normalize, for one finished head — deferred by one head."""
                zrbox = {}

                def zjob():
                    ps_z = ps.tile([1, 512], F32, tag="psD", bufs=2,
                                   name=f"psz{oqb}_{h}")
                    nc.tensor.matmul(ps_z, ones_col, zacc[:, h],
                                     start=True, stop=True)
                    zf = zp.tile([1, 512], F32, tag="zf", bufs=1)
                    nc.vector.reciprocal_approx_fast(zf, ps_z)
                    zr = zp.tile([1, 512], F16, tag="zr", bufs=2)
                    with nc.allow_low_precision(reason="1/z fp16 bcast"):
                        nc.vector.tensor_copy(zr, zf)
                    zrbox[0] = zr

                def njob():
                    zb = ps.tile([128, 512], F32, tag="psD", bufs=2,
                                 name=f"zb{oqb}_{h}")
                    nc.tensor.matmul(zb, ones_row, zrbox[0],
                                     start=True, stop=True)
                    i16 = h * SB + oqb
                    with nc.allow_low_precision(reason="attn norm bf16"):
                        nc.vector.tensor_tensor(
                            attnT[:, i16], attnT[:, i16], zb,
                            op=mybir.AluOpType.mult,
                        )

                return [zjob, njob]

            def oproj_jobs(oqb):
                """o_proj matmuls in et-PAIRS ping-ponging two PSUM banks
                so consecutive o_proj matmuls never share a bank."""
                jobs = []
                opsum = {}

                def ojob(et, jt):
                    def run():
                        pso = opsum[et % 2]
                        nc.tensor.matmul(
                            pso, wo_m[:, jt, et * 128:(et + 1) * 128],
                            attnT[:, jt * SB + oqb],
                            start=(jt == 0), stop=(jt == HPC - 1),
                        )
                        if jt == HPC - 1:
                            drain(et, pso)
                    return run

                def drain(et, ps_oo):
                    oo = outp.tile([128, 512], BF16, tag="oo", bufs=3)
                    nc.vector.tensor_copy(oo, ps_oo)
                    m0 = oqb * 512
                    nc.sync.dma_start(oT_b[:, et, m0:m0 + 256], oo[:, :256])
                    nc.sync.dma_start(oT_b[:, et, m0 + 256:m0 + 512], oo[:, 256:])

                def mkop(par, et):
                    def run():
                        opsum[par] = ps.tile([128, 512], F32, tag="psC",
                                             bufs=2, name=f"poo{oqb}_{et}")
                    return run

                # o_proj in et pairs ping-ponging psC's two banks
                for pair in range(KT // 2):
                    ea, eb_ = 2 * pair, 2 * pair + 1
                    jobs.append(mkop(0, ea))
                    jobs.append(mkop(1, eb_))
                    for jt in range(HPC):
                        jobs.append(ojob(ea, jt))
                        jobs.append(ojob(eb_, jt))
                return jobs

            pending = []
            njobs_hold = []
            for qb in range(SB):
                nkt = 4 * qb + 4
                zacc = zacc_p.tile([128, HPC, 512], F16, tag="za",
                                   name=f"zacc{qb % 2}")
                for h in range(HPC):
                    # fixed pop rate: a backlog-proportional stride crams
                    # 16+ o_proj matmuls between attention matmuls (2-bank
                    # ping-pong degrades to ~360ns each); excess work just
                    # spills into later query blocks / the tail instead
                    stride = 3 if pending else 0
                    ps_o = ps.tile([128, 512], F32, tag="psB", bufs=2,
                                   name=f"pso{qb}_{h}")

                    def score_exp(kt):
                        # diagonal blocks: columns j < 128*r are fully
                        # masked -> skip them (w = valid width)
                        r = kt - 4 * qb
                        j0 = 128 * r if r > 0 else 0
                        w = 512 - j0
                        qsw_ = slice(qb * 512 + j0, (qb + 1) * 512)
                        ps_s = ps.tile([128, w], F32, tag="psA", bufs=2,
                                       name=f"pss{qb}_{h}_{kt}")
                        nc.tensor.matmul(
                            ps_s, rk[:, h, kt * 128: (kt + 1) * 128],
                            rq[:, h, qsw_], start=True, stop=True,
                        )
                        ext = expp.tile([128, w], BF16, tag="exp")
                        # exp is pre-scaled by 2^-4 so the fp16 z
                        # accumulator cannot overflow; the factor cancels
                        # exactly in AV/z normalization
                        nc.scalar.activation(
                            ext, ps_s, mybir.ActivationFunctionType.Exp,
                            scale=SCALE, bias=expb,
                        )
                        if r >= 0:
                            with nc.allow_low_precision(reason="mask 0/1"):
                                nc.vector.tensor_mul(
                                    ext[:, :128], ext[:, :128], masks_sb
                                )
                        return ext, j0

                    # software pipeline: score/exp one kt ahead of AV, so
                    # the PE never waits on the mask+exp latency chain
                    nxt = score_exp(0)
                    for kt in range(nkt):
                        ext, j0 = nxt
                        if kt + 1 < nkt:
                            nxt = score_exp(kt + 1)
                        nc.tensor.matmul(
                            ps_o[:, j0:512],
                            v_m[:, kt, h * 128: (h + 1) * 128], ext,
                            start=(kt == 0), stop=(kt == nkt - 1),
                        )
                        # softmax denominator on the DVE (kt==0 is always
                        # full width, so plain copy initializes)
                        if kt == 0:
                            nc.vector.tensor_copy(zacc[:, h], ext)
                        else:
                            nc.vector.tensor_add(
                                zacc[:, h, j0:512], zacc[:, h, j0:512], ext
                            )
                        for _ in range(stride):
                            if pending:
                                pending.pop(0)()
                    # drain unnormalized AV rows (frees ps_o)
                    i16 = h * SB + qb
                    nc.vector.tensor_copy(attnT[:, i16], ps_o)
                    # this head's z pops during the NEXT head; its normalize
                    # one head later still, so the 1/z broadcast never waits
                    # on the reciprocal's DVE latency
                    zj, nj = zn_jobs(qb, zacc, h)
                    if h > 0:
                        pending.append(njobs_hold.pop(0))
                    pending.append(zj)
                    njobs_hold.append(nj)
                pending.append(njobs_hold.pop(0))
                pending.extend(oproj_jobs(qb))
            while pending:            # final query block's normalize + o_proj
                pending.pop(0)()

    nc.compile()
    return nc


def _get_nc():
    if "S" not in _CACHE:
        _CACHE["S"] = build()
    return _CACHE["S"]


def _causal_mask_templates():
    # 0/1 lower-triangle: within the diagonal 128x128 block of every
    # exp tile, key p attends query column j iff p <= j
    p = np.arange(128)[:, None]
    j = np.arange(128)[None, :]
    return np.ascontiguousarray(np.where(p > j, 0.0, 1.0).astype(np.float32))


def _rope_cache_np():
    # mirrors reference._rope_cache in float32
    inv_freq = (1.0 / (BASE ** (np.arange(0, D, 2, dtype=np.float32) / np.float32(D)))).astype(np.float32)
    ratio = (MIN_R + (MAX_R - MIN_R) * (np.arange(H, dtype=np.float32) / np.float32(H))).astype(np.float32)
    t = (np.arange(S, dtype=np.float32)[None, :] / ratio[:, None]).astype(np.float32)
    freqs = (t[:, :, None] * inv_freq[None, None, :]).astype(np.float32)
    emb = np.concatenate([freqs, freqs], axis=-1)
    return np.cos(emb).astype(np.float32), np.sin(emb).astype(np.float32)


def _head_order(hs, Wq, Wk):
    """Exact head-outlier ordering from the last pre-RoPE attention row,
    computed in f64 on the host: srow_h = hs @ (Wk_h^T (Wq_h hs[-1]))."""
    hs64 = hs.astype(np.float64)
    q_last = hs64[-1] @ Wq.T.astype(np.float64)                 # [HID]
    Wk64 = Wk.astype(np.float64)
    Wall = np.empty((HID, H), np.float64)
    for h in range(H):
        rows = slice(h * D, (h + 1) * D)
        Wall[:, h] = Wk64[rows, :].T @ q_last[rows]
    srow = (hs64 @ Wall).T                                      # [H, S]
    sc = srow * SCALE
    m = sc.max(axis=-1, keepdims=True)
    e = np.exp(sc - m)
    aw = e / e.sum(axis=-1, keepdims=True)
    avg = aw.mean(axis=-1, keepdims=True)
    cnt = (aw > 3.0 * avg).sum(axis=-1)
    outlier = (-(cnt / np.float32(S))).astype(np.float32)
    return np.argsort(outlier, kind="stable")


def kernel(hidden_states, position_ids, Wq, Wk, Wv, Wo):
    hs = np.asarray(hidden_states, dtype=np.float32)[0]        # [S, HID]
    pos = np.asarray(position_ids).astype(np.int64)[0]         # [S]
    Wq = np.asarray(Wq, dtype=np.float32)
    Wk = np.asarray(Wk, dtype=np.float32)
    Wv = np.asarray(Wv, dtype=np.float32)
    Wo = np.asarray(Wo, dtype=np.float32)

    # ---- host: head order (exact control flow), permuted RoPE caches ----
    head_order = _head_order(hs, Wq, Wk)
    cos, sin = _rope_cache_np()
    cos_o = cos[head_order][:, pos, :]                         # [H, S, D]
    sin_o = sin[head_order][:, pos, :]
    masks = _causal_mask_templates()
    # rotate_half as a matmul: sw[m] = -x[m+64] (m<64), +x[m-64] (m>=64)
    p2rot_m = np.zeros((128, 128), np.float32)
    p2rot_m[np.arange(64) + 64, np.arange(64)] = -1.0
    p2rot_m[np.arange(64), np.arange(64) + 64] = 1.0
    p2rot_m = p2rot_m.astype(NPBF16)

    hsT = np.ascontiguousarray(hs.T).astype(NPBF16)            # [HID, S] bf16

    nc = _get_nc()
    in_maps = []
    for c in range(NCORES):
        rows = slice(c * JC, (c + 1) * JC)
        ct = np.ascontiguousarray(
            np.concatenate([cos_o[c * HPC + i].T for i in range(HPC)], axis=0)
        )  # [JC, S]
        # plain sin (rotate_half's swap+sign lives in the P2 matmul)
        st = np.concatenate(
            [sin_o[c * HPC + i].T for i in range(HPC)], axis=0
        )
        in_maps.append(
            {
                "hsT": hsT,
                "wqT": np.ascontiguousarray(Wq[rows, :].T).astype(NPBF16),
                "wkT": np.ascontiguousarray(Wk[rows, :].T).astype(NPBF16),
                "wvT": np.ascontiguousarray(Wv[rows, :].T).astype(NPBF16),
                "woT": np.ascontiguousarray(Wo[:, rows].T).astype(NPBF16),
                "cosT": ct.astype(NPBF16),
                "shatT": np.ascontiguousarray(st).astype(NPBF16),
                "masks": masks.astype(NPBF16),
                "p2rot": p2rot_m,
            }
        )
    res = bass_utils.run_bass_kernel_spmd(
        nc, in_maps, core_ids=list(range(NCORES)), trace=TRACE
    )
    if TRACE:
        LAST_PROFILE["S"] = res

    # ---- host: unshard (sum o_proj partials) ----
    acc = np.zeros((HID, S), np.float64)
    for c in range(NCORES):
        acc += res.results[c]["oT"].astype(np.float64)
    return np.ascontiguousarray(acc.T)[None, :, :].astype(np.float32)


# revision 42
# speedup vs baseline: 1.0500x; 1.0360x over previous
"""MsPoE Llama attention on 8 TRN2 NeuronCores (tensor-parallel over heads).

Strategy (v7: single launch, PE-lean attention, ~586us vs 673us baseline)
-------------------------------------------------------------------------
The reference's head-ordering statistic only needs the LAST pre-RoPE
attention row: srow_h = q_last_h . k_h[s]. By associativity,
srow_h = hs @ (Wk_h^T (Wq_h hs[-1])) — ~0.5 GFLOP, computed on the
host in float64 BEFORE launching (verified to reproduce the reference
head_order exactly). With head_order known up-front, the permuted
per-head RoPE cos/sin caches become plain inputs and the whole module
runs in ONE device launch with q/k/v resident in SBUF.

  PSUM discipline (everything below depends on it): 4 tags x 2 banks,
  each tag's ring rotation matching its freeing order per phase.

  Per core (4 heads), all matmul operands bf16:

  1. QK pass: stream hsT once per sb; wq streams, wk is RESIDENT
     (loaded during sb0 — with both streaming, phase 1 was DMA-starved
     at 74% queue occupancy). Accumulate q/k head-blocks in PSUM,
     then drain: 8 psum->bf16 copies FIRST (alternating ACT/DVE), and
     RoPE's rotate_half as a tiny PE permutation matmul (P2, +/-1)
     back into each just-freed bank — a gpsimd half-multiply here ran
     1.3-1.5us/op and serialized the drain, cascading into the sync
     engine's DMA queue via the trig-ring WAR. The next pass's first
     hs chunk is prefetched before the drain so sb boundaries don't
     stall on DMA. RoPE math: dst = qf*cos + (P2 qf)*sin on DVE.
  2. V pass: stream hsT again, natural-layout V into resident
     v_m [128, 16, 512] bf16. wo loads here, into wk's SBUF slot
     (issued after each sb's hs DMAs — never at the head of the sync
     queue, where its wk WAR-wait would block all later DMA issue).
  3. Attention per (qb outer, head inner): scores^T = rk_chunk^T rq
     per 128-key tile; exp on ACT with a 2^-4 prescale (bias), bf16
     out; causal masking is a 0/1 lower-triangle MULTIPLY on the
     diagonal 128-col block of the exp tile (exact: masked entries
     become 0). The softmax denominator accumulates on the DVE into
     fp16 zacc (2-byte ops, cannot overflow thanks to the prescale;
     the prescale cancels in AV/z normalization). AV accumulates
     unnormalized in PSUM.
  4. Deferred job FIFO, popped 3 PE-ops per kt iteration: per-head z
     ones-matmul + reciprocal (one head late), 1/z PE broadcast +
     attnT normalize (two heads late, so zb never waits the recip
     chain), then o_proj matmuls in et-PAIRS ping-ponging psC's two
     banks. A fixed pop rate (not backlog-proportional) keeps o_proj
     matmuls spaced; excess spills into later query blocks/the tail.

  Host: sum the 8 o_proj partials (f64) -> [1, 2048, 4096].
"""

import os
import sys

import numpy as np

for _p in ("/opt/trn_rl_repo", "/root/.axon_site/_ro/trn_rl_repo"):
    if os.path.isdir(_p) and _p not in sys.path:
        sys.path.append(_p)

import concourse.bass as bass  # noqa: E402
import concourse.tile as tile  # noqa: E402
from concourse import bacc, mybir  # noqa: E402
from concourse import bass_utils  # noqa: E402

import ml_dtypes  # noqa: E402

F32 = mybir.dt.float32
BF16 = mybir.dt.bfloat16
F32R = mybir.dt.float32r
F16 = mybir.dt.float16
NPBF16 = ml_dtypes.bfloat16

B, S, HID, H, D = 1, 2048, 4096, 32, 128
NCORES, HPC = 8, 4          # cores, heads per core
JC = HPC * D                # 512: per-core projection width
KT = HID // 128             # 32 contraction tiles
SB = S // 512               # 4 sequence blocks
EB = 4                      # e-tiles per hs-stream DMA
BASE, MIN_R, MAX_R = 10000.0, 1.0, 3.0
SCALE = 1.0 / float(np.sqrt(D))
EXPB = -4.0 * float(np.log(2.0))   # exp pre-scale 2^-4: fp16 z never overflows

_CACHE = {}
TRACE = False          # set True (e.g. from test.py) to profile the launch
LAST_PROFILE = {}      # filled with BassKernelResults when TRACE is on


def build():
    nc = bacc.Bacc("TRN2", target_bir_lowering=False, debug=False, num_devices=NCORES)
    hsT = nc.dram_tensor("hsT", [HID, S], BF16, kind="ExternalInput").ap()
    wqT = nc.dram_tensor("wqT", [HID, JC], BF16, kind="ExternalInput").ap()
    wkT = nc.dram_tensor("wkT", [HID, JC], BF16, kind="ExternalInput").ap()
    wvT = nc.dram_tensor("wvT", [HID, JC], BF16, kind="ExternalInput").ap()
    woT = nc.dram_tensor("woT", [JC, HID], BF16, kind="ExternalInput").ap()
    cosT = nc.dram_tensor("cosT", [JC, S], BF16, kind="ExternalInput").ap()
    shatT = nc.dram_tensor("shatT", [JC, S], BF16, kind="ExternalInput").ap()
    masks = nc.dram_tensor("masks", [128, 128], BF16, kind="ExternalInput").ap()
    p2rot = nc.dram_tensor("p2rot", [128, 128], BF16, kind="ExternalInput").ap()
    oT = nc.dram_tensor("oT", [HID, S], BF16, kind="ExternalOutput").ap()

    hsT_b = hsT.rearrange("(eb g p) s -> p eb g s", p=128, g=EB)   # [128, 8, EB, S]
    wqT_b = wqT.rearrange("(eb g p) j -> p eb g j", p=128, g=EB)   # [128, 8, EB, JC]
    wkT_b = wkT.rearrange("(eb g p) j -> p eb g j", p=128, g=EB)
    wvT_b = wvT.rearrange("(kt p) j -> p kt j", p=128)             # [128, 32, JC]
    woT_b = woT.rearrange("(jt p) e -> p jt e", p=128)             # [128, 4, HID]
    cosT_b = cosT.rearrange("(h p) s -> p h s", p=128)             # [128, 4, S]
    shatT_b = shatT.rearrange("(h p) s -> p h s", p=128)
    oT_b = oT.rearrange("(et p) s -> p et s", p=128)               # [128, 32, S]
    NB = KT // EB

    with tile.TileContext(nc) as tc:
        with (
            tc.tile_pool(name="wv_res", bufs=1) as wv_res,   # wv resident
            tc.tile_pool(name="wkwo", bufs=1) as wkwo,       # wk then wo (shared slot)
            tc.tile_pool(name="wst", bufs=4) as wst,         # wq streamed chunks
            tc.tile_pool(name="trig", bufs=4) as trig,       # cos/shat per-sb stream
            tc.tile_pool(name="big", bufs=4) as big,         # rq/rk/v/attnT resident
            tc.tile_pool(name="hpool", bufs=3) as hpool,     # hs stream tiles
            tc.tile_pool(name="rtmp", bufs=4) as rtmp,       # RoPE f32 temps
            tc.tile_pool(name="expp", bufs=8) as expp,       # exp/qf tiles bf16
            tc.tile_pool(name="small", bufs=1) as small,
            tc.tile_pool(name="zacc_p", bufs=2) as zacc_p,   # z accumulators f32r
            tc.tile_pool(name="zp", bufs=4) as zp,           # 1/z rows
            tc.tile_pool(name="outp", bufs=4) as outp,
            tc.tile_pool(name="ps", bufs=8, space="PSUM") as ps,
        ):
            rq = big.tile([128, HPC, S], BF16, tag="big", name="rq")
            rk = big.tile([128, HPC, S], BF16, tag="big", name="rk")
            wv_m = wv_res.tile([128, KT, JC], BF16, tag="w", name="wv")
            wk_m = wkwo.tile([128, KT, JC], BF16, tag="wk", name="wk")

            p2m = small.tile([128, 128], BF16)
            nc.sync.dma_start(p2m, p2rot)

            # ---- phase 1: QK projections + fused RoPE ----
            pre1 = {}
            pre2 = {}
            for sb in range(SB):
                ss = slice(sb * 512, (sb + 1) * 512)
                # fixed-lifetime PSUM rings: 4 tags x 2 banks. Each tag's
                # rotation order then matches its freeing order in every phase
                ps_q = [ps.tile([128, 512], F32, tag=f"ps{'AB'[i // 2]}",
                                bufs=2, name=f"psq{sb}_{i}")
                        for i in range(HPC)]
                ps_k = [ps.tile([128, 512], F32, tag=f"ps{'CD'[i // 2]}",
                                bufs=2, name=f"psk{sb}_{i}")
                        for i in range(HPC)]
                cos_t = trig.tile([128, HPC, 512], BF16, tag="t", name=f"cos{sb}")
                shat_t = trig.tile([128, HPC, 512], BF16, tag="t", name=f"shat{sb}")
                for eb in range(NB):
                    # half-tile DMAs land on different queues: halves the
                    # transfer latency each chunk's first matmuls gate on
                    if eb == 0 and sb in pre1:
                        hst, wqs = pre1.pop(sb)   # prefetched last sb
                    else:
                        hst = hpool.tile([128, EB, 512], BF16, tag="h")
                        nc.sync.dma_start(hst[:, :EB // 2], hsT_b[:, eb, :EB // 2, ss])
                        wqs = wst.tile([128, EB, JC], BF16, tag="w",
                                       name=f"wqs{sb}_{eb}")
                        nc.sync.dma_start(wqs[:, :EB // 2], wqT_b[:, eb, :EB // 2])
                        nc.sync.dma_start(hst[:, EB // 2:], hsT_b[:, eb, EB // 2:, ss])
                        nc.sync.dma_start(wqs[:, EB // 2:], wqT_b[:, eb, EB // 2:])
                    if sb == 0:
                        nc.sync.dma_start(
                            wk_m[:, eb * EB: eb * EB + EB // 2],
                            wkT_b[:, eb, :EB // 2],
                        )
                        nc.sync.dma_start(
                            wk_m[:, eb * EB + EB // 2: (eb + 1) * EB],
                            wkT_b[:, eb, EB // 2:],
                        )
                    if eb == 0:
                        nc.sync.dma_start(cos_t, cosT_b[:, :, ss])
                        nc.sync.dma_start(shat_t, shatT_b[:, :, ss])
                    if eb == NB - 1:
                        # prefetch the NEXT pass's first chunk NOW, ahead of
                        # the wv/trig transfers and the drain in the sync
                        # queue (for sb3 that's phase 2's first chunk)
                        ss2 = slice(((sb + 1) % SB) * 512,
                                    ((sb + 1) % SB + 1) * 512)
                        hst2 = hpool.tile([128, EB, 512], BF16, tag="h")
                        nc.sync.dma_start(hst2[:, :EB // 2],
                                          hsT_b[:, 0, :EB // 2, ss2])
                        nc.sync.dma_start(hst2[:, EB // 2:],
                                          hsT_b[:, 0, EB // 2:, ss2])
                        if sb + 1 < SB:
                            wqs2 = wst.tile([128, EB, JC], BF16, tag="w",
                                            name=f"wqs{sb + 1}_0")
                            nc.sync.dma_start(wqs2[:, :EB // 2],
                                              wqT_b[:, 0, :EB // 2])
                            nc.sync.dma_start(wqs2[:, EB // 2:],
                                              wqT_b[:, 0, EB // 2:])
                            pre1[sb + 1] = (hst2, wqs2)
                        else:
                            pre2[0] = hst2
                    last = eb == NB - 1
                    for g in range(EB):
                        e = eb * EB + g
                        if last and g == EB - 1:
                            # final chunk: all q matmuls first, so the q
                            # banks' drain overlaps the k matmuls
                            for jt in range(HPC):
                                js = slice(jt * 128, (jt + 1) * 128)
                                nc.tensor.matmul(ps_q[jt], wqs[:, g, js],
                                                 hst[:, g], start=False, stop=True)
                            for jt in range(HPC):
                                js = slice(jt * 128, (jt + 1) * 128)
                                nc.tensor.matmul(ps_k[jt], wk_m[:, e, js],
                                                 hst[:, g], start=False, stop=True)
                        else:
                            for jt in range(HPC):
                                js = slice(jt * 128, (jt + 1) * 128)
                                nc.tensor.matmul(
                                    ps_q[jt], wqs[:, g, js], hst[:, g],
                                    start=(e == 0), stop=False,
                                )
                                nc.tensor.matmul(
                                    ps_k[jt], wk_m[:, e, js], hst[:, g],
                                    start=(e == 0), stop=False,
                                )
                # prefetch one quarter of wv per sb
                pcs = slice(sb * 8, (sb + 1) * 8)
                nc.sync.dma_start(wv_m[:, pcs], wvT_b[:, pcs])
                # Drain ALL 8 PSUM banks first (copies alternate ACT/DVE, in
                # the bank order the next sb's matmuls need). RoPE's
                # rotate_half is a tiny PE permutation matmul (P2, +/-1
                # entries) back into the just-freed bank — the gpsimd
                # half-multiply it replaces ran 1.3-1.5us per op and
                # serialized the whole drain tail.
                chains = []
                cp = 0
                if sb == SB - 1:
                    order = [0, 2, 1, 3]   # frees psA,psB,psC,psD round-robin
                else:
                    order = [0, 1, 2, 3]   # next sb needs q0,k0,q1,k1,...
                for jt in order:
                    for ps_list, dst, qk in ((ps_q, rq, "q"), (ps_k, rk, "k")):
                        qf = expp.tile([128, 512], BF16, tag="exp")
                        if cp % 2 == 0:
                            nc.scalar.copy(qf, ps_list[jt])
                        else:
                            nc.vector.tensor_copy(qf, ps_list[jt])
                        cp += 1
                        tagc = ("AB" if qk == "q" else "CD")[jt // 2]
                        chains.append((qf, dst, jt, tagc, qk))
                swqs = []
                for qf, dst, jt, tagc, qk in chains:
                    swq = ps.tile([128, 512], F32, tag=f"ps{tagc}", bufs=2,
                                  name=f"swq{sb}_{qk}{jt}")
                    nc.tensor.matmul(swq, p2m, qf, start=True, stop=True)
                    swqs.append(swq)
                for (qf, dst, jt, tagc, qk), swq in zip(chains, swqs):
                    tmp = rtmp.tile([128, 512], BF16, tag="rt", bufs=2)
                    with nc.allow_low_precision(reason="rope bf16 temp"):
                        nc.vector.tensor_mul(tmp, swq, shat_t[:, jt])
                    t2 = rtmp.tile([128, 512], BF16, tag="rt_t2", bufs=2)
                    nc.vector.tensor_mul(t2, qf, cos_t[:, jt])
                    with nc.allow_low_precision(reason="rope bf16 store"):
                        nc.vector.tensor_add(dst[:, jt, ss], t2, tmp)

            # ---- constants for attention (loaded behind phase 2's stream) ----
            # 0/1 lower-triangle template: multiplies the diagonal 128-col
            # sub-block of each exp tile (replaces the additive -1e35 mask;
            # exp of an unmasked score is finite, then zeroed exactly)
            masks_sb = small.tile([128, 128], BF16)
            nc.sync.dma_start(masks_sb, masks)
            onesf = rtmp.tile([128, 1], F32, tag="ones", bufs=2)
            nc.vector.memset(onesf, 1.0)
            ones_col = small.tile([128, 1], F16)
            nc.vector.tensor_copy(ones_col, onesf)
            onesf_r = rtmp.tile([1, 128], F32, tag="ones", bufs=2)
            nc.vector.memset(onesf_r, 1.0)
            ones_row = small.tile([1, 128], F16)
            nc.vector.tensor_copy(ones_row, onesf_r)
            expb = small.tile([128, 1], F32)
            nc.vector.memset(expb, EXPB)

            # ---- phase 2: V projection (natural layout); wo loads into wk's
            # slot (wk is dead after phase 1) ----
            wo_m = wkwo.tile([128, HPC, HID], BF16, tag="wk", name="wo")
            v_m = big.tile([128, S // 128, JC], BF16, tag="big", name="v")
            for sb in range(SB):
                ss = slice(sb * 512, (sb + 1) * 512)
                ps_v = [ps.tile([128, 512], F32, tag=f"ps{'ABCD'[i]}",
                                bufs=2, name=f"psv{sb}_{i}")
                        for i in range(4)]
                for eb in range(NB):
                    if eb == 0 and sb in pre2:
                        hst = pre2.pop(sb)
                    else:
                        hst = hpool.tile([128, EB, 512], BF16, tag="h")
                        nc.sync.dma_start(hst[:, :EB // 2],
                                          hsT_b[:, eb, :EB // 2, ss])
                        nc.sync.dma_start(hst[:, EB // 2:],
                                          hsT_b[:, eb, EB // 2:, ss])
                    if eb == NB - 1 and sb + 1 < SB:
                        ss2 = slice((sb + 1) * 512, (sb + 2) * 512)
                        hst2 = hpool.tile([128, EB, 512], BF16, tag="h")
                        nc.sync.dma_start(hst2[:, :EB // 2],
                                          hsT_b[:, 0, :EB // 2, ss2])
                        nc.sync.dma_start(hst2[:, EB // 2:],
                                          hsT_b[:, 0, EB // 2:, ss2])
                        pre2[sb + 1] = hst2
                    for g in range(EB):
                        e = eb * EB + g
                        for t4 in range(4):
                            cs = slice(t4 * 128, (t4 + 1) * 128)
                            nc.tensor.matmul(
                                ps_v[t4], hst[:, g, cs], wv_m[:, e],
                                start=(e == 0), stop=(e == KT - 1),
                            )
                # one quarter of wo per sb, issued AFTER this sb's hst
                # dma_starts so they never queue behind the wk WAR wait
                nc.sync.dma_start(wo_m[:, :, sb * 1024:(sb + 1) * 1024],
                                  woT_b[:, :, sb * 1024:(sb + 1) * 1024])
                for t4 in range(4):
                    if t4 % 2 == 0:
                        nc.scalar.copy(v_m[:, sb * 4 + t4], ps_v[t4])
                    else:
                        nc.vector.tensor_copy(v_m[:, sb * 4 + t4], ps_v[t4])

            # ---- phase 3+4: attention (qb outer) + deferred normalize/o_proj
            attnT = big.tile([128, HPC * SB, 512], BF16, tag="big", name="attnT")
            oo_flip = 0

            def zn_jobs(oqb, zacc, h):
                """z partition-sum + reciprocal, then 1/z broadcast +
                normalize, for one finished head — deferred by one head."""
                zrbox = {}

                def zjob():
                    ps_z = ps.tile([1, 512], F32, tag="psD", bufs=2,
                                   name=f"psz{oqb}_{h}")
                    nc.tensor.matmul(ps_z, ones_col, zacc[:, h],
                                     start=True, stop=True)
                    zf = zp.tile([1, 512], F32, tag="zf", bufs=1)
                    nc.vector.reciprocal_approx_fast(zf, ps_z)
                    zr = zp.tile([1, 512], F16, tag="zr", bufs=2)
                    with nc.allow_low_precision(reason="1/z fp16 bcast"):
                        nc.vector.tensor_copy(zr, zf)
                    zrbox[0] = zr

                def njob():
                    zb = ps.tile([128, 512], F32, tag="psD", bufs=2,
                                 name=f"zb{oqb}_{h}")
                    nc.tensor.matmul(zb, ones_row, zrbox[0],
                                     start=True, stop=True)
                    i16 = h * SB + oqb
                    with nc.allow_low_precision(reason="attn norm bf16"):
                        nc.vector.tensor_tensor(
                            attnT[:, i16], attnT[:, i16], zb,
                            op=mybir.AluOpType.mult,
                        )

                return [zjob, njob]

            def oproj_jobs(oqb):
                """o_proj matmuls in et-PAIRS ping-ponging two PSUM banks
                so consecutive o_proj matmuls never share a bank."""
                jobs = []
                opsum = {}

                def ojob(et, jt):
                    def run():
                        pso = opsum[et % 2]
                        nc.tensor.matmul(
                            pso, wo_m[:, jt, et * 128:(et + 1) * 128],
                            attnT[:, jt * SB + oqb],
                            start=(jt == 0), stop=(jt == HPC - 1),
                        )
                        if jt == HPC - 1:
                            drain(et, pso)
                    return run

                oo_box = {}

                def drain(et, ps_oo):
                    # drain an et-PAIR into one buffer and issue a SINGLE
                    # oT dma_start per pair: each dma_start costs ~600ns on
                    # the serial sync engine, which throttled the tail
                    if et % 2 == 0:
                        oo_box[0] = outp.tile([128, 2, 512], BF16, tag="oo",
                                              bufs=2, name="oo_pair")
                        nc.vector.tensor_copy(oo_box[0][:, 0], ps_oo)
                    else:
                        oo = oo_box[0]
                        nc.vector.tensor_copy(oo[:, 1], ps_oo)
                        m0 = oqb * 512
                        nc.sync.dma_start(oT_b[:, et - 1:et + 1, m0:m0 + 512],
                                          oo)

                def mkop(par, et):
                    def run():
                        opsum[par] = ps.tile([128, 512], F32, tag="psC",
                                             bufs=2, name=f"poo{oqb}_{et}")
                    return run

                # o_proj in et pairs ping-ponging psC's two banks
                for pair in range(KT // 2):
                    ea, eb_ = 2 * pair, 2 * pair + 1
                    jobs.append(mkop(0, ea))
                    jobs.append(mkop(1, eb_))
                    for jt in range(HPC):
                        jobs.append(ojob(ea, jt))
                        jobs.append(ojob(eb_, jt))
                return jobs

            pending = []
            njobs_hold = []
            for qb in range(SB):
                nkt = 4 * qb + 4
                zacc = zacc_p.tile([128, HPC, 512], F16, tag="za",
                                   name=f"zacc{qb % 2}")
                for h in range(HPC):
                    # fixed pop rate: a backlog-proportional stride crams
                    # 16+ o_proj matmuls between attention matmuls (2-bank
                    # ping-pong degrades to ~360ns each); excess work just
                    # spills into later query blocks / the tail instead
                    stride = 3 if pending else 0
                    ps_o = ps.tile([128, 512], F32, tag="psB", bufs=2,
                                   name=f"pso{qb}_{h}")

                    def score_exp(kt):
                        # diagonal blocks: columns j < 128*r are fully
                        # masked -> skip them (w = valid width)
                        r = kt - 4 * qb
                        j0 = 128 * r if r > 0 else 0
                        w = 512 - j0
                        qsw_ = slice(qb * 512 + j0, (qb + 1) * 512)
                        ps_s = ps.tile([128, w], F32, tag="psA", bufs=2,
                                       name=f"pss{qb}_{h}_{kt}")
                        nc.tensor.matmul(
                            ps_s, rk[:, h, kt * 128: (kt + 1) * 128],
                            rq[:, h, qsw_], start=True, stop=True,
                        )
                        ext = expp.tile([128, w], BF16, tag="exp")
                        # exp is pre-scaled by 2^-4 so the fp16 z
                        # accumulator cannot overflow; the factor cancels
                        # exactly in AV/z normalization
                        nc.scalar.activation(
                            ext, ps_s, mybir.ActivationFunctionType.Exp,
                            scale=SCALE, bias=expb,
                        )
                        if r >= 0:
                            with nc.allow_low_precision(reason="mask 0/1"):
                                nc.vector.tensor_mul(
                                    ext[:, :128], ext[:, :128], masks_sb
                                )
                        return ext, j0

                    # software pipeline: score/exp one kt ahead of AV, so
                    # the PE never waits on the mask+exp latency chain
                    nxt = score_exp(0)
                    for kt in range(nkt):
                        ext, j0 = nxt
                        if kt + 1 < nkt:
                            nxt = score_exp(kt + 1)
                        nc.tensor.matmul(
                            ps_o[:, j0:512],
                            v_m[:, kt, h * 128: (h + 1) * 128], ext,
                            start=(kt == 0), stop=(kt == nkt - 1),
                        )
                        # softmax denominator on the DVE (kt==0 is always
                        # full width, so plain copy initializes)
                        if kt == 0:
                            nc.vector.tensor_copy(zacc[:, h], ext)
                        else:
                            nc.vector.tensor_add(
                                zacc[:, h, j0:512], zacc[:, h, j0:512], ext
                            )
                        for _ in range(stride):
                            if pending:
                                pending.pop(0)()
                    # drain unnormalized AV rows (frees ps_o)
                    i16 = h * SB + qb
                    nc.vector.tensor_copy(attnT[:, i16], ps_o)
                    # this head's z pops during the NEXT head; its normalize
                    # one head later still, so the 1/z broadcast never waits
                    # on the reciprocal's DVE latency
                    zj, nj = zn_jobs(qb, zacc, h)
                    if h > 0:
                        pending.append(njobs_hold.pop(0))
                    pending.append(zj)
                    njobs_hold.append(nj)
                pending.append(njobs_hold.pop(0))
                pending.extend(oproj_jobs(qb))
            while pending:            # final query block's normalize + o_proj
                pending.pop(0)()

    nc.compile()
    return nc


def _get_nc():
    if "S" not in _CACHE:
        _CACHE["S"] = build()
    return _CACHE["S"]


def _causal_mask_templates():
    # 0/1 lower-triangle: within the diagonal 128x128 block of every
    # exp tile, key p attends query column j iff p <= j
    p = np.arange(128)[:, None]
    j = np.arange(128)[None, :]
    return np.ascontiguousarray(np.where(p > j, 0.0, 1.0).astype(np.float32))


def _rope_cache_np():
    # mirrors reference._rope_cache in float32
    inv_freq = (1.0 / (BASE ** (np.arange(0, D, 2, dtype=np.float32) / np.float32(D)))).astype(np.float32)
    ratio = (MIN_R + (MAX_R - MIN_R) * (np.arange(H, dtype=np.float32) / np.float32(H))).astype(np.float32)
    t = (np.arange(S, dtype=np.float32)[None, :] / ratio[:, None]).astype(np.float32)
    freqs = (t[:, :, None] * inv_freq[None, None, :]).astype(np.float32)
    emb = np.concatenate([freqs, freqs], axis=-1)
    return np.cos(emb).astype(np.float32), np.sin(emb).astype(np.float32)


def _head_order(hs, Wq, Wk):
    """Exact head-outlier ordering from the last pre-RoPE attention row,
    computed in f64 on the host: srow_h = hs @ (Wk_h^T (Wq_h hs[-1]))."""
    hs64 = hs.astype(np.float64)
    q_last = hs64[-1] @ Wq.T.astype(np.float64)                 # [HID]
    Wk64 = Wk.astype(np.float64)
    Wall = np.empty((HID, H), np.float64)
    for h in range(H):
        rows = slice(h * D, (h + 1) * D)
        Wall[:, h] = Wk64[rows, :].T @ q_last[rows]
    srow = (hs64 @ Wall).T                                      # [H, S]
    sc = srow * SCALE
    m = sc.max(axis=-1, keepdims=True)
    e = np.exp(sc - m)
    aw = e / e.sum(axis=-1, keepdims=True)
    avg = aw.mean(axis=-1, keepdims=True)
    cnt = (aw > 3.0 * avg).sum(axis=-1)
    outlier = (-(cnt / np.float32(S))).astype(np.float32)
    return np.argsort(outlier, kind="stable")


def kernel(hidden_states, position_ids, Wq, Wk, Wv, Wo):
    hs = np.asarray(hidden_states, dtype=np.float32)[0]        # [S, HID]
    pos = np.asarray(position_ids).astype(np.int64)[0]         # [S]
    Wq = np.asarray(Wq, dtype=np.float32)
    Wk = np.asarray(Wk, dtype=np.float32)
    Wv = np.asarray(Wv, dtype=np.float32)
    Wo = np.asarray(Wo, dtype=np.float32)

    # ---- host: head order (exact control flow), permuted RoPE caches ----
    head_order = _head_order(hs, Wq, Wk)
    cos, sin = _rope_cache_np()
    cos_o = cos[head_order][:, pos, :]                         # [H, S, D]
    sin_o = sin[head_order][:, pos, :]
    masks = _causal_mask_templates()
    # rotate_half as a matmul: sw[m] = -x[m+64] (m<64), +x[m-64] (m>=64)
    p2rot_m = np.zeros((128, 128), np.float32)
    p2rot_m[np.arange(64) + 64, np.arange(64)] = -1.0
    p2rot_m[np.arange(64), np.arange(64) + 64] = 1.0
    p2rot_m = p2rot_m.astype(NPBF16)

    hsT = np.ascontiguousarray(hs.T).astype(NPBF16)            # [HID, S] bf16

    nc = _get_nc()
    in_maps = []
    for c in range(NCORES):
        rows = slice(c * JC, (c + 1) * JC)
        ct = np.ascontiguousarray(
            np.concatenate([cos_o[c * HPC + i].T for i in range(HPC)], axis=0)
        )  # [JC, S]
        # plain sin (rotate_half's swap+sign lives in the P2 matmul)
        st = np.concatenate(
            [sin_o[c * HPC + i].T for i in range(HPC)], axis=0
        )
        in_maps.append(
            {
                "hsT": hsT,
                "wqT": np.ascontiguousarray(Wq[rows, :].T).astype(NPBF16),
                "wkT": np.ascontiguousarray(Wk[rows, :].T).astype(NPBF16),
                "wvT": np.ascontiguousarray(Wv[rows, :].T).astype(NPBF16),
                "woT": np.ascontiguousarray(Wo[:, rows].T).astype(NPBF16),
                "cosT": ct.astype(NPBF16),
                "shatT": np.ascontiguousarray(st).astype(NPBF16),
                "masks": masks.astype(NPBF16),
                "p2rot": p2rot_m,
            }
        )
    res = bass_utils.run_bass_kernel_spmd(
        nc, in_maps, core_ids=list(range(NCORES)), trace=TRACE
    )
    if TRACE:
        LAST_PROFILE["S"] = res

    # ---- host: unshard (sum o_proj partials) ----
    acc = np.zeros((HID, S), np.float64)
    for c in range(NCORES):
        acc += res.results[c]["oT"].astype(np.float64)
    return np.ascontiguousarray(acc.T)[None, :, :].astype(np.float32)
